# revision 2
# baseline (speedup 1.0000x reference)
"""Trainium2 Bass kernel for nn_LinearInFieldChargesBlock (bf16, transposed-layout, DMA-superbatched).

Math (per node n):
  out0[w] = 0.01*(C000 * sum_{u,v} x0[u] y0[v] w000[u,v,w]
                 + C110 * sum_{u,v,i} x1[u,i] y1[v,i] w110[u,v,w])
  out1[w,k] = 0.01*(C011 * sum_{u,v} x0[u] y1[v,k] w011[u,v,w]
                 + C101 * sum_{u,v} x1[u,k] y0[v] w101[u,v,w])
  out = concat([out0, out1.reshape(-1)]) with column 0 zeroed.

Kernel formulation (all weights host-precomputed; node_feat/pot_feat are
fed to the device pre-transposed so features sit on SBUF partitions and
no on-device transposes are needed):
  P1 = W1^T X0t [64, n]   P2 = W2^T X1t [96, n]   (PE, contract u)
  Yb = B^T Yt (0/1 selector broadcasts y values per feature)  (PE)
  Q = P * Yb                                       (vector elementwise)
  out_t = R1^T Q1 + R2^T Q2 (0/1 reduction, col 0 zeroed)  (PE)
out_t [16, n] is DMA'd straight from PSUM to HBM and un-transposed on host.

Sharding: pure data-parallel across 8 cores along the node axis; the tiny
path-weight matrices are replicated.
"""

import sys

import numpy as np

try:
    import concourse  # noqa: F401
except ImportError:
    sys.path.insert(0, "/opt/trn_rl_repo")

N_NODES = 400000
N_CORES = 8
BATCH = 512  # nodes per pipeline batch
SUPER = 7  # batches per DMA super-batch (amortizes per-DMA fixed costs)
CHUNK = BATCH * SUPER  # 3584 nodes per DMA instruction
PER_CORE = 50176  # ceil(400000/8) rounded up to a multiple of CHUNK
PADDED = PER_CORE * N_CORES

_F32R = True  # run PE matmuls in float32r (fast fp32 mode)


def _build_mats(w000, w011, w101, w110):
    C000 = 1 / 32.0
    C110 = 1 / (32.0 * np.sqrt(3.0))
    C011 = 1 / 32.0
    C101 = 1 / 32.0
    S = 0.01
    W1 = np.zeros((128, 64), np.float32)
    for v in range(4):
        for w in range(4):
            W1[:, v * 4 + w] = S * C000 * w000[:, v, w]
            for k in range(3):
                W1[:, 16 + (v * 4 + w) * 3 + k] = S * C011 * w011[:, v, w]
    W2 = np.zeros((384, 96), np.float32)
    u_idx = np.arange(384) // 3
    for i in range(3):
        rows = np.arange(384)[np.arange(384) % 3 == i]
        for v in range(4):
            for w in range(4):
                W2[rows, (v * 4 + w) * 3 + i] = S * C110 * w110[u_idx[rows], v, w]
                W2[rows, 48 + (v * 4 + w) * 3 + i] = S * C101 * w101[u_idx[rows], v, w]
    B1 = np.zeros((16, 64), np.float32)
    R1 = np.zeros((64, 16), np.float32)
    B2 = np.zeros((16, 96), np.float32)
    R2 = np.zeros((96, 16), np.float32)
    for v in range(4):
        for w in range(4):
            B1[v, v * 4 + w] = 1
            if w > 0:
                R1[v * 4 + w, w] = 1
            for k in range(3):
                B1[4 + v * 3 + k, 16 + (v * 4 + w) * 3 + k] = 1
                R1[16 + (v * 4 + w) * 3 + k, 4 + w * 3 + k] = 1
            for i in range(3):
                B2[4 + v * 3 + i, (v * 4 + w) * 3 + i] = 1
                if w > 0:
                    R2[(v * 4 + w) * 3 + i, w] = 1
                B2[v, 48 + (v * 4 + w) * 3 + i] = 1
                R2[48 + (v * 4 + w) * 3 + i, 4 + w * 3 + i] = 1
    return W1, W2, B1, B2, R1, R2


_CACHE = {}


def build_kernel(n_nodes):
    """Build + compile the per-core Bass program for n_nodes (multiple of 512)."""
    if n_nodes in _CACHE:
        return _CACHE[n_nodes]

    import concourse.bacc as bacc
    import concourse.tile as tile
    from concourse import mybir

    f32 = mybir.dt.float32
    mm_dt = mybir.dt.bfloat16

    nc = bacc.Bacc(None, target_bir_lowering=False)
    xtd = nc.dram_tensor("xt", [512, n_nodes], mm_dt, kind="ExternalInput")
    ytd = nc.dram_tensor("yt", [16, n_nodes], mm_dt, kind="ExternalInput")
    # all path weights packed into one [128, 544] tensor (single preamble DMA):
    # cols 0:64 w1 | 64+96c w2 chunk c | 352:416 b1 (rows 0:16) |
    # 416:512 b2 (rows 0:16) | 512:528 r1 (rows 0:64) | 528:544 r2 (rows 0:96)
    wpd = nc.dram_tensor("wpack", [128, 544], mm_dt, kind="ExternalInput")
    outd = nc.dram_tensor("out_t", [16, n_nodes], mm_dt, kind="ExternalOutput")

    nbatches = n_nodes // BATCH

    with tile.TileContext(nc) as tc:
        with (
            tc.tile_pool(name="consts", bufs=1) as consts,
            tc.tile_pool(name="xin", bufs=3) as xin,
            tc.tile_pool(name="yin", bufs=3) as yin,
            tc.tile_pool(name="ybs", bufs=4) as ybsp,
            tc.tile_pool(name="q", bufs=4) as qp,
            tc.tile_pool(name="osb", bufs=3) as osbp,
            tc.tile_pool(name="p1ps", bufs=2, space="PSUM") as p1psp,
            tc.tile_pool(name="p2ps", bufs=2, space="PSUM") as p2psp,
            tc.tile_pool(name="ybps", bufs=2, space="PSUM") as ybpsp,
            tc.tile_pool(name="ops", bufs=2, space="PSUM") as opsp,
        ):
            wall = consts.tile([128, 544], mm_dt, tag="wpack")
            nc.sync.dma_start(out=wall[:], in_=wpd[:])
            w1s = wall[:, 0:64]
            w2s = [wall[:, 64 + 96 * c : 64 + 96 * (c + 1)] for c in range(3)]
            b1s = wall[0:16, 352:416]
            b2s = wall[0:16, 416:512]
            r1s = wall[0:64, 512:528]
            r2s = wall[0:96, 528:544]

            def emit_out(prev_q1, prev_q2, prev_osb, prev_s, prev_k):
                # reduce Q features -> 16 outputs (transposed), stage in SBUF
                otq = opsp.tile([16, BATCH], f32, tag="ot")
                nc.tensor.matmul(otq[:], r1s, prev_q1[:], start=True, stop=False)
                nc.tensor.matmul(otq[:], r2s, prev_q2[:], start=False, stop=True)
                nc.scalar.copy(
                    out=prev_osb[:, prev_k * BATCH : (prev_k + 1) * BATCH],
                    in_=otq[:],
                )
                if prev_k == SUPER - 1:
                    # out DMA on Pool's software DGE: keeps SP free for X
                    nc.gpsimd.dma_start(
                        out=outd[:, prev_s * CHUNK : (prev_s + 1) * CHUNK],
                        in_=prev_osb[:],
                    )

            nsupers = nbatches // SUPER

            def load_y(s):
                # Y DMA on Pool's software DGE (prefetched one super-batch
                # ahead), keeping SP free for X and ACT free for copies
                Yt = yin.tile([16, CHUNK], mm_dt, tag="y", name=f"Y{s}")
                nc.gpsimd.dma_start(
                    out=Yt[:], in_=ytd[:, s * CHUNK : (s + 1) * CHUNK]
                )
                return Yt

            prev = None
            X = Y = osb = None
            Ynext = load_y(0)
            for b in range(nbatches):
                s, k = divmod(b, SUPER)
                if k == 0:
                    # one DMA instruction per super-batch for X / Y / out
                    # X layout: partition = feature-within-chunk,
                    # free = (chunk c, node m within super-batch)
                    X = xin.tile([128, 4 * CHUNK], mm_dt, tag="x")
                    nc.sync.dma_start(
                        out=X[:].rearrange("p (c m) -> p c m", c=4),
                        in_=xtd[:, s * CHUNK : (s + 1) * CHUNK].rearrange(
                            "(c p) m -> p c m", p=128
                        ),
                    )
                    Y = Ynext
                    if s + 1 < nsupers:
                        Ynext = load_y(s + 1)
                    osb = osbp.tile([16, CHUNK], mm_dt, tag="osb")
                nsl = slice(k * BATCH, (k + 1) * BATCH)
                # broadcast y values per P-feature (issued first: only needs Y)
                yb1 = ybpsp.tile([64, BATCH], f32, tag="yb")
                nc.tensor.matmul(yb1[:], b1s, Y[:, nsl], start=True, stop=True)
                yb2 = ybpsp.tile([96, BATCH], f32, tag="yb")
                nc.tensor.matmul(yb2[:], b2s, Y[:, nsl], start=True, stop=True)
                # contraction over u -> P features [feat, node]
                p1 = p1psp.tile([64, BATCH], f32, tag="p1")
                nc.tensor.matmul(
                    p1[:], w1s, X[:, k * BATCH : (k + 1) * BATCH],
                    start=True, stop=True,
                )
                p2 = p2psp.tile([96, BATCH], f32, tag="p2")
                for c in range(3):
                    nc.tensor.matmul(
                        p2[:],
                        w2s[c],
                        X[:, (c + 1) * CHUNK + k * BATCH : (c + 1) * CHUNK + (k + 1) * BATCH],
                        start=(c == 0),
                        stop=(c == 2),
                    )
                ybs1 = ybsp.tile([64, BATCH], f32, tag="ybs")
                nc.scalar.copy(out=ybs1[:], in_=yb1[:])
                ybs2 = ybsp.tile([96, BATCH], f32, tag="ybs")
                nc.scalar.copy(out=ybs2[:], in_=yb2[:])
                # Q = P * Ybroad
                q1 = qp.tile([64, BATCH], mm_dt, tag="q")
                nc.vector.tensor_mul(q1[:], p1[:], ybs1[:])
                q2 = qp.tile([96, BATCH], mm_dt, tag="q")
                nc.vector.tensor_mul(q2[:], p2[:], ybs2[:])
                # output reduction of the PREVIOUS batch (software pipelining:
                # keeps PE from head-of-line blocking on this batch's q tiles)
                if prev is not None:
                    emit_out(*prev)
                prev = (q1, q2, osb, s, k)
            emit_out(*prev)
    nc.compile()
    _CACHE[n_nodes] = nc
    return nc


def make_in_maps(node_feat, pot_feat, w000, w011, w101, w110, per_core=None):
    """Host-side prep: shard along nodes, pre-transpose X/Y, cast to bf16."""
    import ml_dtypes

    bf16 = ml_dtypes.bfloat16
    node_feat = np.asarray(node_feat, dtype=np.float32)
    pot_feat = np.asarray(pot_feat, dtype=np.float32)
    n = node_feat.shape[0]
    if per_core is None:
        per_core = -(-n // (N_CORES * CHUNK)) * CHUNK
    W1, W2, B1, B2, R1, R2 = _build_mats(
        np.asarray(w000, np.float32),
        np.asarray(w011, np.float32),
        np.asarray(w101, np.float32),
        np.asarray(w110, np.float32),
    )
    wpack = np.zeros((128, 544), np.float32)
    wpack[:, 0:64] = W1
    for c in range(3):
        wpack[:, 64 + 96 * c : 64 + 96 * (c + 1)] = W2[c * 128 : (c + 1) * 128]
    wpack[0:16, 352:416] = B1
    wpack[0:16, 416:512] = B2
    wpack[0:64, 512:528] = R1
    wpack[0:96, 528:544] = R2
    wpack = wpack.astype(bf16)
    in_maps = []
    for i in range(N_CORES):
        lo = min(i * per_core, n)
        hi = min((i + 1) * per_core, n)
        xt = np.zeros((512, per_core), bf16)
        yt = np.zeros((16, per_core), bf16)
        if hi > lo:
            xt[:, : hi - lo] = node_feat[lo:hi].T.astype(bf16)
            yt[:, : hi - lo] = pot_feat[lo:hi].T.astype(bf16)
        in_maps.append({"xt": xt, "yt": yt, "wpack": wpack})
    return in_maps, per_core, n


def kernel(node_feat, pot_feat, w000, w011, w101, w110, **extra_kwargs):
    from concourse.bass_utils import run_bass_kernel_spmd

    in_maps, per_core, n = make_in_maps(
        node_feat, pot_feat, w000, w011, w101, w110
    )
    nc = build_kernel(per_core)
    res = run_bass_kernel_spmd(nc, in_maps, core_ids=list(range(N_CORES)))
    out = np.empty((N_CORES * per_core, 16), np.float32)
    for i in range(N_CORES):
        out[i * per_core : (i + 1) * per_core] = (
            res.results[i]["out_t"].astype(np.float32).T
        )
    return out[:n]
